# revision 25
# baseline (speedup 1.0000x reference)
"""Ragged single-query attention with shared k-projection, on 8 trn2 cores.

Math (algebraically identical to the reference, reassociated):
    enc_k = k @ Wk.T ; enc_q = Wk @ q ; scores = enc_k @ enc_q
           = k @ (Wk.T @ (Wk @ q)) = k @ gq          <- no enc_k needed
    E = masked_softmax(scores)
    attended = (v @ Wk.T).T-weighted sum with E
             = Wk @ (E @ v) = vbar @ Wk.T            <- no enc_v needed

This turns 275 GFLOP of projections into ~0.3 GFLOP of matvecs plus a
single streaming pass over k and v (the memory-bound part that actually
has to happen). Sharding: data-parallel over batch, 4 batches per core,
Wk (both layouts) replicated.

On-device per core (4 batches b, 16 s-tiles t of 128 rows each):
    gq     = Wk.T @ (Wk @ q_b)                   (PE, tiny)
    scores = per-s-tile fused mul+reduce k*gq    (DVE tensor_tensor_reduce)
    E      = exact two-level masked softmax      (DVE/ACT/PE-transpose)
    vbar   = sum_t E_col(t)^T (x) v_tile(t)      (PE, E scattered to 4 lanes)
    att    = vbar @ Wk.T                         (PE)
"""

import os
import sys
import types

import numpy as np

B, S, D = 32, 2048, 1024
NCORES = 8
BPC = B // NCORES  # batches per core: 4
T = S // 128  # s-tiles per batch: 16
NEG = -1.0e30

_CACHE = {}


# --------------------------------------------------------------------------
# workaround: the neuronxcc in this container rejects instructions carrying
# more than one sync-wait ("Too many sync wait commands") — at least for
# LDWEIGHTS and CTRL-class ops, and Tile's tail drain waits on the full
# global clock. Hoist all but the last wait of every multi-wait instruction
# onto injected single-wait NoOps just before it on the same engine queue.
# The queue executes in order and blocks on each wait, so this is
# semantically identical.
def _split_multiwait_drains(nc):
    import bass_rust
    import concourse.mybir as mybir

    for f in nc.m.functions:
        for blk in f.blocks:
            new_insts = []
            for inst in blk.instructions:
                si = inst.sync_info
                if si is not None and len(si.on_wait) > 1:
                    waits = list(si.on_wait)
                    for j, w in enumerate(waits[:-1]):
                        d = mybir.InstNoOp(
                            name=f"{inst.name}_sw{j}",
                            sync_info=bass_rust.SyncInfo(
                                on_wait=[w], on_update=[]
                            ),
                            bass_nofuse=True,
                            engine=inst.engine,
                        )
                        new_insts.append(d)
                    si.on_wait = [waits[-1]]
                    inst.sync_info = si
                new_insts.append(inst)
            blk.instructions[:] = new_insts


def _build_nc():
    from contextlib import ExitStack

    import concourse.bass as bass
    import concourse.tile as tile
    from concourse import mybir
    from concourse.masks import make_identity

    f32 = mybir.dt.float32
    f32r = mybir.dt.float32r
    nc = bass.Bass()

    k_in = nc.dram_tensor("k_in", [BPC * S, D], f32, kind="ExternalInput")
    v_in = nc.dram_tensor("v_in", [BPC * S, D], f32r, kind="ExternalInput")
    q_in = nc.dram_tensor("q_in", [BPC, D], f32, kind="ExternalInput")
    wk = nc.dram_tensor("wk", [D, D], f32, kind="ExternalInput")
    wkt = nc.dram_tensor("wkt", [D, D], f32, kind="ExternalInput")
    mask_in = nc.dram_tensor("mask_in", [128, BPC * T], f32, kind="ExternalInput")
    e_out = nc.dram_tensor("e_out", [BPC, S], f32, kind="ExternalOutput")
    att_out = nc.dram_tensor("att_out", [BPC, D], f32, kind="ExternalOutput")

    NT = BPC * T  # total s-tiles per core (64)

    with tile.TileContext(nc) as tc:
        with ExitStack() as ctx:
            ep = ctx.enter_context  # shorthand

            consts = ep(tc.tile_pool(name="consts", bufs=1))
            kpool = ep(tc.tile_pool(name="kpool", bufs=10))
            vpool = ep(tc.tile_pool(name="vpool", bufs=10))
            smalls = ep(tc.tile_pool(name="smalls", bufs=4))
            ps_t = ep(tc.tile_pool(name="ps_t", bufs=2, space="PSUM"))
            ps_mm = ep(tc.tile_pool(name="ps_mm", bufs=2, space="PSUM"))
            ps_vb = ep(tc.tile_pool(name="ps_vb", bufs=1, space="PSUM"))

            # ---- constants / weights in SBUF ----
            # rings: sync = k stream only; scalar = wkt chunks then v
            # stream; gpsimd = q/mask/wk + softmax smalls + e_out.
            # Per-chunk weight tiles so consumers start as chunks land.
            q_sb = consts.tile([BPC, D], f32, tag="q_sb")
            nc.gpsimd.dma_start(q_sb[:], q_in[:])
            mask_sb = consts.tile([128, NT], f32, tag="mask_sb")
            nc.gpsimd.dma_start(mask_sb[:], mask_in[:])
            wkt_c = []
            for j in range(8):
                wt_j = consts.tile([128, D], f32, tag=f"wkt{j}", name=f"wkt{j}")
                nc.scalar.dma_start(wt_j[:], wkt[128 * j : 128 * (j + 1), :])
                wkt_c.append(wt_j)
            wk_c = []
            for j in range(8):
                wj = consts.tile([128, D], f32, tag=f"wk{j}", name=f"wk{j}")
                nc.gpsimd.dma_start(wj[:], wk[128 * j : 128 * (j + 1), :])
                wk_c.append(wj)

            ident = consts.tile([128, 128], f32, tag="ident")
            make_identity(nc, ident[:])
            ones_row = consts.tile([1, 128], f32, tag="ones_row")
            nc.vector.memset(ones_row[:], 1.0)
            # selmat[b, 128*b + i] = 1, else 0 — batch-row selector for the
            # gq replication matmuls (PE rhs must sit at base partition 0).
            selmat = consts.tile([BPC, 128 * BPC], f32, tag="selmat")
            nc.gpsimd.memset(selmat[:], 1.0)
            nc.gpsimd.affine_select(
                out=selmat[:],
                in_=selmat[:],
                compare_op=mybir.AluOpType.is_ge,
                fill=0.0,
                base=0,
                channel_multiplier=-128,
                pattern=[[1, 128 * BPC]],
            )
            nc.gpsimd.affine_select(
                out=selmat[:],
                in_=selmat[:],
                compare_op=mybir.AluOpType.is_ge,
                fill=0.0,
                base=127,
                channel_multiplier=128,
                pattern=[[-1, 128 * BPC]],
            )

            # ---- qT tiles: q [4, D] -> qT [128, 4] per d-tile ----
            qT = consts.tile([128, 8 * BPC], f32, tag="qT")
            for j in range(8):
                pt = ps_t.tile([128, BPC], f32, tag="tp")
                nc.tensor.transpose(
                    pt[:], q_sb[:, 128 * j : 128 * (j + 1)], ident[0:BPC, 0:BPC]
                )
                nc.scalar.copy(qT[:, BPC * j : BPC * (j + 1)], pt[:])

            # ---- qh = q @ Wk.T  ([4, D], contraction over d) ----
            qh_sb = consts.tile([BPC, D], f32, tag="qh_sb")
            for c in range(2):
                pm = ps_mm.tile([BPC, 512], f32, tag="mm")
                for j in range(8):
                    nc.tensor.matmul(
                        pm[:],
                        qT[:, BPC * j : BPC * (j + 1)],
                        wkt_c[j][:, 512 * c : 512 * (c + 1)],
                        start=(j == 0),
                        stop=(j == 7),
                    )
                nc.scalar.copy(qh_sb[:, 512 * c : 512 * (c + 1)], pm[:])

            # ---- qhT tiles ----
            qhT = consts.tile([128, 8 * BPC], f32, tag="qhT")
            for j in range(8):
                pt = ps_t.tile([128, BPC], f32, tag="tp")
                nc.tensor.transpose(
                    pt[:], qh_sb[:, 128 * j : 128 * (j + 1)], ident[0:BPC, 0:BPC]
                )
                nc.scalar.copy(qhT[:, BPC * j : BPC * (j + 1)], pt[:])

            # ---- gq = qh @ Wk  ([4, D], contraction over p) ----
            gq_sb = consts.tile([BPC, D], f32, tag="gq_sb")
            for c in range(2):
                pm = ps_mm.tile([BPC, 512], f32, tag="mm")
                for j in range(8):
                    nc.tensor.matmul(
                        pm[:],
                        qhT[:, BPC * j : BPC * (j + 1)],
                        wk_c[j][:, 512 * c : 512 * (c + 1)],
                        start=(j == 0),
                        stop=(j == 7),
                    )
                nc.scalar.copy(gq_sb[:, 512 * c : 512 * (c + 1)], pm[:])

            # ---- gq replicated to 128 partitions, per batch ----
            gqrep = consts.tile([128, BPC * D], f32, tag="gqrep")
            for b in range(BPC):
                for c in range(2):
                    pr = ps_mm.tile([128, 512], f32, tag="mm")
                    nc.tensor.matmul(
                        pr[:],
                        selmat[:, 128 * b : 128 * (b + 1)],
                        gq_sb[:, 512 * c : 512 * (c + 1)],
                        start=True,
                        stop=True,
                    )
                    nc.scalar.copy(
                        gqrep[:, D * b + 512 * c : D * b + 512 * (c + 1)], pr[:]
                    )

            # ---- per-batch: stream k -> scores -> softmax -> vbar ----
            # Separate tiles per batch so Tile's whole-tile dependency
            # tracking doesn't serialize batch pipelines against each other.
            vb0 = ps_vb.tile([BPC, 512], f32, tag="vb0")
            vb1 = ps_vb.tile([BPC, 512], f32, tag="vb1")
            rall = consts.tile([1, BPC], f32, tag="rall")
            eb4s = []
            for b in range(BPC):
                eb4_t = consts.tile([128, T * BPC], f32r, tag=f"eb4_{b}", name=f"eb4_{b}")
                eb4s.append(eb4_t)
                nc.vector.memset(eb4s[b][:].bitcast(f32), 0.0)
            for b in range(BPC):
                scores = smalls.tile([128, T], f32, tag=f"scores{b}")
                for t in range(T):
                    it = b * T + t
                    kt = kpool.tile([128, D], f32, tag="kt")
                    nc.sync.dma_start(kt[:], k_in[128 * it : 128 * (it + 1), :])
                    # k *= gq on DVE; reduce along d on ACT (fused into a copy)
                    nc.vector.tensor_mul(
                        kt[:], kt[:], gqrep[:, D * b : D * (b + 1)]
                    )
                    nc.scalar.activation(
                        out=kt[:],
                        in_=kt[:],
                        func=mybir.ActivationFunctionType.Copy,
                        bias=0.0,
                        scale=1.0,
                        accum_out=scores[:, t : t + 1],
                    )

                smask = smalls.tile([128, T], f32, tag=f"smask{b}")
                nc.vector.tensor_add(
                    smask[:],
                    scores[:],
                    mask_sb[:, T * b : T * (b + 1)],
                )
                # cross-partition max/sum via tiny SBUF->SBUF DMAs on the
                # (otherwise idle) gpsimd queue — keeps PE out of the
                # softmax chain so batch pipelines overlap on the PE FIFO.
                m_p = smalls.tile([128, 1], f32, tag="m_p")
                nc.vector.tensor_reduce(
                    out=m_p[:],
                    in_=smask[:],
                    axis=mybir.AxisListType.X,
                    op=mybir.AluOpType.max,
                )
                mT = smalls.tile([1, 128], f32, tag="mT")
                nc.gpsimd.dma_start(mT[:], m_p[:])
                negM = smalls.tile([1, 1], f32, tag="negM")
                nc.vector.tensor_reduce(
                    out=negM[:],
                    in_=mT[:],
                    axis=mybir.AxisListType.X,
                    op=mybir.AluOpType.max,
                    negate=True,
                )
                negMb = smalls.tile([128, 1], f32, tag="negMb")
                nc.gpsimd.dma_start(negMb[:], negM[0:1, 0:1].to_broadcast((1, 128)))
                et = smalls.tile([128, T], f32, tag="et")
                se_p = smalls.tile([128, 1], f32, tag="se_p")
                nc.scalar.activation(
                    out=et[:],
                    in_=smask[:],
                    func=mybir.ActivationFunctionType.Exp,
                    bias=negMb[:],
                    scale=1.0,
                    accum_out=se_p[:],
                )
                seT = smalls.tile([1, 128], f32, tag="seT")
                nc.gpsimd.dma_start(seT[:], se_p[:])
                tot = smalls.tile([1, 1], f32, tag="tot")
                nc.vector.tensor_reduce(
                    out=tot[:],
                    in_=seT[:],
                    axis=mybir.AxisListType.X,
                    op=mybir.AluOpType.add,
                )
                # reciprocal lands in rall[0, b]; vbar row b is scaled by it
                # at the end (keeps normalization off the vbar critical path)
                nc.vector.reciprocal(rall[0:1, b : b + 1], tot[:])
                rb = smalls.tile([128, 1], f32, tag="rb")
                nc.gpsimd.dma_start(
                    rb[:], rall[0:1, b : b + 1].to_broadcast((1, 128))
                )
                # E = exp(s - M) / tot   (fully normalized, output only)
                e_sb = smalls.tile([128, T], f32, tag="e_sb")
                nc.vector.tensor_scalar_mul(e_sb[:], et[:], rb[:])
                nc.gpsimd.dma_start(
                    e_out[b, :].rearrange("(t p) -> p t", p=128), e_sb[:]
                )
                # per-batch 4-lane weights (UNNORMALIZED): lane b = exp
                # columns, others 0 — vbar matmuls depend only on this
                # batch's exp, not on the sum/reciprocal chain
                eb4_3d = eb4s[b][:].rearrange("p (t four) -> p t four", four=BPC)
                nc.vector.tensor_copy(
                    eb4_3d[:, :, b : b + 1],
                    et[:].rearrange("p (t o) -> p t o", o=1),
                )
                # vbar accumulation for this batch's 16 v tiles
                for t in range(T):
                    it = b * T + t
                    vt = vpool.tile([128, D], f32r, tag="vt")
                    nc.scalar.dma_start(vt[:], v_in[128 * it : 128 * (it + 1), :])
                    nc.tensor.matmul(
                        vb0[:],
                        eb4_3d[:, t, :],
                        vt[:, 0:512],
                        start=(it == 0),
                        stop=(it == NT - 1),
                    )
                    nc.tensor.matmul(
                        vb1[:],
                        eb4_3d[:, t, :],
                        vt[:, 512:1024],
                        start=(it == 0),
                        stop=(it == NT - 1),
                    )

            vbar = consts.tile([BPC, D], f32, tag="vbar")
            nc.scalar.copy(vbar[:, 0:512], vb0[:])
            nc.scalar.copy(vbar[:, 512:1024], vb1[:])
            # normalize: row b *= 1/sum_b
            r4 = consts.tile([BPC, 1], f32, tag="r4")
            nc.gpsimd.dma_start(r4[:], rall[:])
            nc.vector.tensor_scalar_mul(vbar[:], vbar[:], r4[:])

            # ---- vbarT tiles ----
            vbT = consts.tile([128, 8 * BPC], f32, tag="vbT")
            for j in range(8):
                pt = ps_t.tile([128, BPC], f32, tag="tp")
                nc.tensor.transpose(
                    pt[:], vbar[:, 128 * j : 128 * (j + 1)], ident[0:BPC, 0:BPC]
                )
                nc.scalar.copy(vbT[:, BPC * j : BPC * (j + 1)], pt[:])

            # ---- attended = vbar @ Wk.T ----
            att_sb = consts.tile([BPC, D], f32, tag="att_sb")
            for c in range(2):
                pm = ps_mm.tile([BPC, 512], f32, tag="mm")
                for j in range(8):
                    nc.tensor.matmul(
                        pm[:],
                        vbT[:, BPC * j : BPC * (j + 1)],
                        wkt_c[j][:, 512 * c : 512 * (c + 1)],
                        start=(j == 0),
                        stop=(j == 7),
                    )
                nc.scalar.copy(att_sb[:, 512 * c : 512 * (c + 1)], pm[:])
            nc.sync.dma_start(att_out[:], att_sb[:])

    _split_multiwait_drains(nc)
    return nc


def _get_nc():
    if "nc" not in _CACHE:
        _CACHE["nc"] = _build_nc()
    return _CACHE["nc"]


def kernel(k, q, v, k_lens, Wk):
    from concourse.bass_utils import run_bass_kernel_spmd

    k = np.ascontiguousarray(k, dtype=np.float32)
    v = np.ascontiguousarray(v, dtype=np.float32)
    q = np.ascontiguousarray(q, dtype=np.float32)
    Wk = np.ascontiguousarray(Wk, dtype=np.float32)
    k_lens = np.asarray(k_lens)
    wkt = np.ascontiguousarray(Wk.T)

    # mask[p, b*T + t] = 0 if (t*128+p) < len_b else NEG
    s_idx = np.arange(S).reshape(T, 128)
    in_maps = []
    for c in range(NCORES):
        lens = k_lens[BPC * c : BPC * (c + 1)].astype(np.int64)
        m3 = np.where(s_idx[None, :, :] < lens[:, None, None], 0.0, NEG)
        mask = np.transpose(m3, (2, 0, 1)).reshape(128, BPC * T)
        in_maps.append(
            {
                "k_in": k[BPC * c : BPC * (c + 1)].reshape(BPC * S, D),
                "v_in": v[BPC * c : BPC * (c + 1)].reshape(BPC * S, D),
                "q_in": q[BPC * c : BPC * (c + 1)],
                "wk": Wk,
                "wkt": wkt,
                "mask_in": np.ascontiguousarray(mask, dtype=np.float32),
            }
        )

    nc = _get_nc()
    res = run_bass_kernel_spmd(nc, in_maps, core_ids=list(range(NCORES)))

    att = np.empty((B, D), dtype=np.float32)
    E = np.empty((B, S), dtype=np.float32)
    for c in range(NCORES):
        att[BPC * c : BPC * (c + 1)] = res.results[c]["att_out"]
        E[BPC * c : BPC * (c + 1)] = res.results[c]["e_out"]
    return att, E[:, :, None]


# revision 27
# speedup vs baseline: 1.1913x; 1.1913x over previous
"""Ragged single-query attention with shared k-projection, on 8 trn2 cores.

Math (algebraically identical to the reference, reassociated):
    enc_k = k @ Wk.T ; enc_q = Wk @ q ; scores = enc_k @ enc_q
           = k @ (Wk.T @ (Wk @ q)) = k @ gq          <- no enc_k needed
    E = masked_softmax(scores)
    attended = (v @ Wk.T).T-weighted sum with E
             = Wk @ (E @ v) = vbar @ Wk.T            <- no enc_v needed

This turns 275 GFLOP of projections into ~0.3 GFLOP of matvecs plus a
single streaming pass over k and v (the memory-bound part that actually
has to happen). Sharding: data-parallel over batch, 4 batches per core,
Wk (both layouts) replicated.

On-device per core (4 batches b, 16 s-tiles t of 128 rows each):
    gq     = Wk.T @ (Wk @ q_b)                   (PE, tiny)
    scores = per-s-tile fused mul+reduce k*gq    (DVE tensor_tensor_reduce)
    E      = exact two-level masked softmax      (DVE/ACT/PE-transpose)
    vbar   = sum_t E_col(t)^T (x) v_tile(t)      (PE, E scattered to 4 lanes)
    att    = vbar @ Wk.T                         (PE)
"""

import os
import sys
import types

import numpy as np

B, S, D = 32, 2048, 1024
NCORES = 8
BPC = B // NCORES  # batches per core: 4
T = S // 128  # s-tiles per batch: 16
NEG = -1.0e30

_CACHE = {}


# --------------------------------------------------------------------------
# workaround: the neuronxcc in this container rejects instructions carrying
# more than one sync-wait ("Too many sync wait commands") — at least for
# LDWEIGHTS and CTRL-class ops, and Tile's tail drain waits on the full
# global clock. Hoist all but the last wait of every multi-wait instruction
# onto injected single-wait NoOps just before it on the same engine queue.
# The queue executes in order and blocks on each wait, so this is
# semantically identical.
def _split_multiwait_drains(nc):
    import bass_rust
    import concourse.mybir as mybir

    for f in nc.m.functions:
        for blk in f.blocks:
            new_insts = []
            for inst in blk.instructions:
                si = inst.sync_info
                if si is not None and len(si.on_wait) > 1:
                    waits = list(si.on_wait)
                    for j, w in enumerate(waits[:-1]):
                        d = mybir.InstNoOp(
                            name=f"{inst.name}_sw{j}",
                            sync_info=bass_rust.SyncInfo(
                                on_wait=[w], on_update=[]
                            ),
                            bass_nofuse=True,
                            engine=inst.engine,
                        )
                        new_insts.append(d)
                    si.on_wait = [waits[-1]]
                    inst.sync_info = si
                new_insts.append(inst)
            blk.instructions[:] = new_insts


def _build_nc():
    from contextlib import ExitStack

    import concourse.bass as bass
    import concourse.tile as tile
    from concourse import mybir
    from concourse.masks import make_identity

    f32 = mybir.dt.float32
    f32r = mybir.dt.float32r
    nc = bass.Bass()

    k_in = nc.dram_tensor("k_in", [BPC * S, D], f32, kind="ExternalInput")
    v_in = nc.dram_tensor("v_in", [BPC * S, D], f32r, kind="ExternalInput")
    q_in = nc.dram_tensor("q_in", [BPC, D], f32, kind="ExternalInput")
    wk = nc.dram_tensor("wk", [D, D], f32, kind="ExternalInput")
    wkt = nc.dram_tensor("wkt", [D, D], f32, kind="ExternalInput")
    mask_in = nc.dram_tensor("mask_in", [128, BPC * T], f32, kind="ExternalInput")
    e_out = nc.dram_tensor("e_out", [BPC, S], f32, kind="ExternalOutput")
    att_out = nc.dram_tensor("att_out", [BPC, D], f32, kind="ExternalOutput")

    NT = BPC * T  # total s-tiles per core (64)

    with tile.TileContext(nc) as tc:
        with ExitStack() as ctx:
            ep = ctx.enter_context  # shorthand

            consts = ep(tc.tile_pool(name="consts", bufs=1))
            kpool = ep(tc.tile_pool(name="kpool", bufs=10))
            vpool = ep(tc.tile_pool(name="vpool", bufs=10))
            smalls = ep(tc.tile_pool(name="smalls", bufs=4))
            ps_t = ep(tc.tile_pool(name="ps_t", bufs=2, space="PSUM"))
            ps_mm = ep(tc.tile_pool(name="ps_mm", bufs=2, space="PSUM"))
            ps_vb = ep(tc.tile_pool(name="ps_vb", bufs=1, space="PSUM"))

            # ---- constants / weights in SBUF ----
            # rings: sync = k stream only; scalar = wkt chunks then v
            # stream; gpsimd = q/mask/wk + softmax smalls + e_out.
            # Per-chunk weight tiles so consumers start as chunks land.
            q_sb = consts.tile([BPC, D], f32, tag="q_sb")
            nc.gpsimd.dma_start(q_sb[:], q_in[:])
            mask_sb = consts.tile([128, NT], f32, tag="mask_sb")
            nc.gpsimd.dma_start(mask_sb[:], mask_in[:])
            wkt_c = []
            for j in range(8):
                wt_j = consts.tile([128, D], f32, tag=f"wkt{j}", name=f"wkt{j}")
                nc.scalar.dma_start(wt_j[:], wkt[128 * j : 128 * (j + 1), :])
                wkt_c.append(wt_j)
            wk_c = []
            for j in range(8):
                wj = consts.tile([128, D], f32, tag=f"wk{j}", name=f"wk{j}")
                nc.gpsimd.dma_start(wj[:], wk[128 * j : 128 * (j + 1), :])
                wk_c.append(wj)

            ident = consts.tile([128, 128], f32, tag="ident")
            make_identity(nc, ident[:])
            ones_row = consts.tile([1, 128], f32, tag="ones_row")
            nc.vector.memset(ones_row[:], 1.0)
            # selmat[b, 128*b + i] = 1, else 0 — batch-row selector for the
            # gq replication matmuls (PE rhs must sit at base partition 0).
            selmat = consts.tile([BPC, 128 * BPC], f32, tag="selmat")
            nc.gpsimd.memset(selmat[:], 1.0)
            nc.gpsimd.affine_select(
                out=selmat[:],
                in_=selmat[:],
                compare_op=mybir.AluOpType.is_ge,
                fill=0.0,
                base=0,
                channel_multiplier=-128,
                pattern=[[1, 128 * BPC]],
            )
            nc.gpsimd.affine_select(
                out=selmat[:],
                in_=selmat[:],
                compare_op=mybir.AluOpType.is_ge,
                fill=0.0,
                base=127,
                channel_multiplier=128,
                pattern=[[-1, 128 * BPC]],
            )

            # ---- qT tiles: q [4, D] -> qT [128, 4] per d-tile ----
            qT = consts.tile([128, 8 * BPC], f32, tag="qT")
            for j in range(8):
                pt = ps_t.tile([128, BPC], f32, tag="tp")
                nc.tensor.transpose(
                    pt[:], q_sb[:, 128 * j : 128 * (j + 1)], ident[0:BPC, 0:BPC]
                )
                nc.scalar.copy(qT[:, BPC * j : BPC * (j + 1)], pt[:])

            # ---- qh = q @ Wk.T  ([4, D], contraction over d) ----
            qh_sb = consts.tile([BPC, D], f32, tag="qh_sb")
            for c in range(2):
                pm = ps_mm.tile([BPC, 512], f32, tag="mm")
                for j in range(8):
                    nc.tensor.matmul(
                        pm[:],
                        qT[:, BPC * j : BPC * (j + 1)],
                        wkt_c[j][:, 512 * c : 512 * (c + 1)],
                        start=(j == 0),
                        stop=(j == 7),
                    )
                nc.scalar.copy(qh_sb[:, 512 * c : 512 * (c + 1)], pm[:])

            # ---- qhT tiles ----
            qhT = consts.tile([128, 8 * BPC], f32, tag="qhT")
            for j in range(8):
                pt = ps_t.tile([128, BPC], f32, tag="tp")
                nc.tensor.transpose(
                    pt[:], qh_sb[:, 128 * j : 128 * (j + 1)], ident[0:BPC, 0:BPC]
                )
                nc.scalar.copy(qhT[:, BPC * j : BPC * (j + 1)], pt[:])

            # ---- gq = qh @ Wk  ([4, D], contraction over p) ----
            gq_sb = consts.tile([BPC, D], f32, tag="gq_sb")
            for c in range(2):
                pm = ps_mm.tile([BPC, 512], f32, tag="mm")
                for j in range(8):
                    nc.tensor.matmul(
                        pm[:],
                        qhT[:, BPC * j : BPC * (j + 1)],
                        wk_c[j][:, 512 * c : 512 * (c + 1)],
                        start=(j == 0),
                        stop=(j == 7),
                    )
                nc.scalar.copy(gq_sb[:, 512 * c : 512 * (c + 1)], pm[:])

            # ---- gq replicated to 128 partitions, per batch ----
            gqrep = consts.tile([128, BPC * D], f32, tag="gqrep")
            for b in range(BPC):
                for c in range(2):
                    pr = ps_mm.tile([128, 512], f32, tag="mm")
                    nc.tensor.matmul(
                        pr[:],
                        selmat[:, 128 * b : 128 * (b + 1)],
                        gq_sb[:, 512 * c : 512 * (c + 1)],
                        start=True,
                        stop=True,
                    )
                    nc.scalar.copy(
                        gqrep[:, D * b + 512 * c : D * b + 512 * (c + 1)], pr[:]
                    )

            # ---- per-batch: stream k -> scores -> softmax -> vbar ----
            # Separate tiles per batch so Tile's whole-tile dependency
            # tracking doesn't serialize batch pipelines against each other.
            vb0 = ps_vb.tile([BPC, 512], f32, tag="vb0")
            vb1 = ps_vb.tile([BPC, 512], f32, tag="vb1")
            rall = consts.tile([1, BPC], f32, tag="rall")
            eb4s = []
            for b in range(BPC):
                eb4_t = consts.tile([128, T * BPC], f32r, tag=f"eb4_{b}", name=f"eb4_{b}")
                eb4s.append(eb4_t)
                nc.vector.memset(eb4s[b][:].bitcast(f32), 0.0)
            scores_by_b = {}

            def emit_scores(b):
                scores = smalls.tile(
                    [128, T], f32, tag=f"scores{b}", name=f"scores{b}"
                )
                scores_by_b[b] = scores
                for t in range(T):
                    it = b * T + t
                    kt = kpool.tile([128, D], f32, tag="kt", name="kt")
                    nc.sync.dma_start(kt[:], k_in[128 * it : 128 * (it + 1), :])
                    # k *= gq on DVE; reduce along d on ACT (fused into a copy)
                    nc.vector.tensor_mul(
                        kt[:], kt[:], gqrep[:, D * b : D * (b + 1)]
                    )
                    nc.scalar.activation(
                        out=kt[:],
                        in_=kt[:],
                        func=mybir.ActivationFunctionType.Copy,
                        bias=0.0,
                        scale=1.0,
                        accum_out=scores[:, t : t + 1],
                    )

            def emit_softmax_vbar(b):
                scores = scores_by_b[b]
                smask = smalls.tile([128, T], f32, tag=f"smask{b}", name=f"smask{b}")
                nc.vector.tensor_add(
                    smask[:],
                    scores[:],
                    mask_sb[:, T * b : T * (b + 1)],
                )
                # cross-partition max/sum via tiny SBUF->SBUF DMAs on the
                # (otherwise idle) gpsimd queue — keeps PE out of the
                # softmax chain so batch pipelines overlap on the PE FIFO.
                m_p = smalls.tile([128, 1], f32, tag="m_p")
                nc.vector.tensor_reduce(
                    out=m_p[:],
                    in_=smask[:],
                    axis=mybir.AxisListType.X,
                    op=mybir.AluOpType.max,
                )
                mT = smalls.tile([1, 128], f32, tag="mT")
                nc.gpsimd.dma_start(mT[:], m_p[:])
                negM = smalls.tile([1, 1], f32, tag="negM")
                nc.vector.tensor_reduce(
                    out=negM[:],
                    in_=mT[:],
                    axis=mybir.AxisListType.X,
                    op=mybir.AluOpType.max,
                    negate=True,
                )
                negMb = smalls.tile([128, 1], f32, tag="negMb")
                nc.gpsimd.dma_start(negMb[:], negM[0:1, 0:1].to_broadcast((1, 128)))
                et = smalls.tile([128, T], f32, tag="et")
                se_p = smalls.tile([128, 1], f32, tag="se_p")
                nc.scalar.activation(
                    out=et[:],
                    in_=smask[:],
                    func=mybir.ActivationFunctionType.Exp,
                    bias=negMb[:],
                    scale=1.0,
                    accum_out=se_p[:],
                )
                seT = smalls.tile([1, 128], f32, tag="seT")
                nc.gpsimd.dma_start(seT[:], se_p[:])
                tot = smalls.tile([1, 1], f32, tag="tot")
                nc.vector.tensor_reduce(
                    out=tot[:],
                    in_=seT[:],
                    axis=mybir.AxisListType.X,
                    op=mybir.AluOpType.add,
                )
                # reciprocal lands in rall[0, b]; vbar row b is scaled by it
                # at the end (keeps normalization off the vbar critical path)
                nc.vector.reciprocal(rall[0:1, b : b + 1], tot[:])
                rb = smalls.tile([128, 1], f32, tag="rb")
                nc.gpsimd.dma_start(
                    rb[:], rall[0:1, b : b + 1].to_broadcast((1, 128))
                )
                # E = exp(s - M) / tot   (fully normalized, output only)
                e_sb = smalls.tile([128, T], f32, tag="e_sb")
                nc.vector.tensor_scalar_mul(e_sb[:], et[:], rb[:])
                nc.gpsimd.dma_start(
                    e_out[b, :].rearrange("(t p) -> p t", p=128), e_sb[:]
                )
                # per-batch 4-lane weights (UNNORMALIZED): lane b = exp
                # columns, others 0 — vbar matmuls depend only on this
                # batch's exp, not on the sum/reciprocal chain
                eb4_3d = eb4s[b][:].rearrange("p (t four) -> p t four", four=BPC)
                nc.vector.tensor_copy(
                    eb4_3d[:, :, b : b + 1],
                    et[:].rearrange("p (t o) -> p t o", o=1),
                )
                # vbar accumulation for this batch's 16 v tiles
                for t in range(T):
                    it = b * T + t
                    vt = vpool.tile([128, D], f32r, tag="vt")
                    nc.scalar.dma_start(vt[:], v_in[128 * it : 128 * (it + 1), :])
                    nc.tensor.matmul(
                        vb0[:],
                        eb4_3d[:, t, :],
                        vt[:, 0:512],
                        start=(it == 0),
                        stop=(it == NT - 1),
                    )
                    nc.tensor.matmul(
                        vb1[:],
                        eb4_3d[:, t, :],
                        vt[:, 512:1024],
                        start=(it == 0),
                        stop=(it == NT - 1),
                    )

            # software-pipeline emission by one batch: batch b's softmax
            # (whose DVE ops block on gpsimd DMA round-trips) sits BEHIND
            # batch b+1's score muls in the engine FIFOs, so the round
            # trips complete while the DVE chews the next batch.
            for b in range(BPC):
                emit_scores(b)
                if b >= 1:
                    emit_softmax_vbar(b - 1)
            emit_softmax_vbar(BPC - 1)

            vbar = consts.tile([BPC, D], f32, tag="vbar")
            nc.scalar.copy(vbar[:, 0:512], vb0[:])
            nc.scalar.copy(vbar[:, 512:1024], vb1[:])
            # normalize: row b *= 1/sum_b
            r4 = consts.tile([BPC, 1], f32, tag="r4")
            nc.gpsimd.dma_start(r4[:], rall[:])
            nc.vector.tensor_scalar_mul(vbar[:], vbar[:], r4[:])

            # ---- vbarT tiles ----
            vbT = consts.tile([128, 8 * BPC], f32, tag="vbT")
            for j in range(8):
                pt = ps_t.tile([128, BPC], f32, tag="tp")
                nc.tensor.transpose(
                    pt[:], vbar[:, 128 * j : 128 * (j + 1)], ident[0:BPC, 0:BPC]
                )
                nc.scalar.copy(vbT[:, BPC * j : BPC * (j + 1)], pt[:])

            # ---- attended = vbar @ Wk.T ----
            att_sb = consts.tile([BPC, D], f32, tag="att_sb")
            for c in range(2):
                pm = ps_mm.tile([BPC, 512], f32, tag="mm")
                for j in range(8):
                    nc.tensor.matmul(
                        pm[:],
                        vbT[:, BPC * j : BPC * (j + 1)],
                        wkt_c[j][:, 512 * c : 512 * (c + 1)],
                        start=(j == 0),
                        stop=(j == 7),
                    )
                nc.scalar.copy(att_sb[:, 512 * c : 512 * (c + 1)], pm[:])
            nc.sync.dma_start(att_out[:], att_sb[:])

    _split_multiwait_drains(nc)
    return nc


def _get_nc():
    if "nc" not in _CACHE:
        _CACHE["nc"] = _build_nc()
    return _CACHE["nc"]


def kernel(k, q, v, k_lens, Wk):
    from concourse.bass_utils import run_bass_kernel_spmd

    k = np.ascontiguousarray(k, dtype=np.float32)
    v = np.ascontiguousarray(v, dtype=np.float32)
    q = np.ascontiguousarray(q, dtype=np.float32)
    Wk = np.ascontiguousarray(Wk, dtype=np.float32)
    k_lens = np.asarray(k_lens)
    wkt = np.ascontiguousarray(Wk.T)

    # mask[p, b*T + t] = 0 if (t*128+p) < len_b else NEG
    s_idx = np.arange(S).reshape(T, 128)
    in_maps = []
    for c in range(NCORES):
        lens = k_lens[BPC * c : BPC * (c + 1)].astype(np.int64)
        m3 = np.where(s_idx[None, :, :] < lens[:, None, None], 0.0, NEG)
        mask = np.transpose(m3, (2, 0, 1)).reshape(128, BPC * T)
        in_maps.append(
            {
                "k_in": k[BPC * c : BPC * (c + 1)].reshape(BPC * S, D),
                "v_in": v[BPC * c : BPC * (c + 1)].reshape(BPC * S, D),
                "q_in": q[BPC * c : BPC * (c + 1)],
                "wk": Wk,
                "wkt": wkt,
                "mask_in": np.ascontiguousarray(mask, dtype=np.float32),
            }
        )

    nc = _get_nc()
    res = run_bass_kernel_spmd(nc, in_maps, core_ids=list(range(NCORES)))

    att = np.empty((B, D), dtype=np.float32)
    E = np.empty((B, S), dtype=np.float32)
    for c in range(NCORES):
        att[BPC * c : BPC * (c + 1)] = res.results[c]["att_out"]
        E[BPC * c : BPC * (c + 1)] = res.results[c]["e_out"]
    return att, E[:, :, None]
